# Initial kernel scaffold
#
"""TRN2 Bass kernel for nn_CosClassifier: sim = 10*scalar * cos_sim(inputs, proto).

Data-parallel over 8 NeuronCores: each core computes a (2048, 4096) slab of the
(16384, 4096) similarity matrix. Per core:
  1. DMA in x-slab (2048,256), proto (4096,256), scalar.
  2. Row norms via ACT Square+accum -> Sqrt; DVE reciprocal.
  3. Scale x rows by 10/||x||, proto rows by scalar/||p|| (DVE tensor_scalar).
  4. PE-transpose scaled operands 128x128-blockwise; the PSUM->SBUF copy
     casts to float32r (TF32-like) so the main matmul runs at 1 cycle/row.
  5. dots matmul in fp32r, accumulated fp32 in PSUM; PSUM->SBUF copies
     alternate between ACT and DVE; 2MB contiguous row-block DMAs out.
"""
import sys

sys.path.insert(0, "/opt/trn_rl_repo")

import numpy as np

B, C, D = 16384, 4096, 256
NCORES = 8
BS = B // NCORES          # 2048 rows per core
NB = BS // 128            # 16 b-tiles per core
NCT = C // 128            # 32 c-tiles (proto rows)
NK = D // 128             # 2 k-tiles
NN = C // 512             # 8 n-blocks of 512

_compiled = None


def _build():
    import concourse.bacc as bacc
    import concourse.mybir as mybir
    import concourse.tile as tile

    f32 = mybir.dt.float32
    f32r = mybir.dt.float32r

    nc = bacc.Bacc("TRN2", target_bir_lowering=False, debug=False,
                   num_devices=NCORES)

    x_d = nc.dram_tensor("x", [BS, D], f32, kind="ExternalInput").ap()
    p_d = nc.dram_tensor("proto", [C, D], f32, kind="ExternalInput").ap()
    s_d = nc.dram_tensor("scalar", [1, 1], f32, kind="ExternalInput").ap()
    id_d = nc.dram_tensor("identity", [128, 128], f32, kind="ExternalInput").ap()
    out_d = nc.dram_tensor("out", [BS, C], f32, kind="ExternalOutput").ap()

    with tile.TileContext(nc) as tc:
        with tc.tile_pool(name="sbuf", bufs=1) as pool, \
             tc.tile_pool(name="outp", bufs=2) as outp, \
             tc.tile_pool(name="psum_t", bufs=2, space="PSUM") as psum_t, \
             tc.tile_pool(name="psum_m", bufs=6, space="PSUM") as psum_m:

            ident = pool.tile([128, 128], f32, tag="ident")
            nc.sync.dma_start(ident[:], id_d[:, :])

            sc = pool.tile([1, 1], f32, tag="sc")
            nc.sync.dma_start(sc[:], s_d[:, :])
            sc_b = pool.tile([128, 1], f32, tag="sc_b")
            nc.gpsimd.partition_broadcast(sc_b[:], sc[:])

            # ---- load x slab: [128, NB*256], b-tile i at cols i*256 ----
            x_s = pool.tile([128, NB * D], f32, tag="x_s")
            x_r = x_d.rearrange("(n p) d -> p n d", p=128)       # [128, NB, 256]
            nc.sync.dma_start(x_s[:].rearrange("p (n d) -> p n d", d=D), x_r)

            # ---- load proto: [128, NCT*256], c-tile j at cols j*256 ----
            p_s = pool.tile([128, NCT * D], f32, tag="p_s")
            p_r = p_d.rearrange("(n p) d -> p n d", p=128)       # [128, NCT, 256]
            for h in range(4):  # 4 DMAs of 8 c-tiles each for earlier start
                nc.sync.dma_start(
                    x_sl := p_s[:, h * 8 * D:(h + 1) * 8 * D].rearrange(
                        "p (n d) -> p n d", d=D),
                    p_r[h * 8:(h + 1) * 8, :, :].rearrange("n p d -> p n d"))

            # ---- norms + scaling ----
            sq_scratch = pool.tile([128, D], f32, tag="sq_scratch")

            def row_scale(src_slice, i, with_scalar):
                ssq = pool.tile([128, 1], f32, tag=f"ssq_{with_scalar}_{i % 2}")
                nc.scalar.activation(sq_scratch[:], src_slice,
                                     mybir.ActivationFunctionType.Square,
                                     accum_out=ssq[:])
                nrm = pool.tile([128, 1], f32, tag=f"nrm_{with_scalar}_{i % 2}")
                # norm/10 for x (folds the *10); plain norm for proto
                scale = 0.01 if not with_scalar else 1.0
                nc.scalar.activation(nrm[:], ssq[:],
                                     mybir.ActivationFunctionType.Sqrt,
                                     scale=scale)
                inv = pool.tile([128, 1], f32, tag=f"inv_{with_scalar}_{i % 2}")
                nc.vector.reciprocal(inv[:], nrm[:])
                if with_scalar:
                    nc.vector.tensor_mult(inv[:], inv[:], sc_b[:])
                nc.vector.tensor_scalar_mul(src_slice, src_slice, inv[:])

            for i in range(NB):
                row_scale(x_s[:, i * D:(i + 1) * D], i, False)
            for j in range(NCT):
                row_scale(p_s[:, j * D:(j + 1) * D], j, True)

            # ---- transposes (PE) with cast to f32r in PSUM->SBUF copy ----
            # xt: [128, NK*BS] f32r, k-block k at cols k*BS, b-tile i at +i*128
            xt = pool.tile([128, NK * BS], f32r, tag="xt")
            # pt: [128, NK*C] f32r, k-block k at cols k*C, c-tile j at +j*128
            pt = pool.tile([128, NK * C], f32r, tag="pt")

            for i in range(NB):
                for k in range(NK):
                    tp = psum_t.tile([128, 128], f32, tag="tp")
                    nc.tensor.transpose(
                        tp[:], x_s[:, i * D + k * 128: i * D + (k + 1) * 128],
                        ident[:])
                    nc.vector.tensor_copy(
                        xt[:, k * BS + i * 128: k * BS + (i + 1) * 128], tp[:])
            for j in range(NCT):
                for k in range(NK):
                    tp = psum_t.tile([128, 128], f32, tag="tp")
                    nc.tensor.transpose(
                        tp[:], p_s[:, j * D + k * 128: j * D + (k + 1) * 128],
                        ident[:])
                    nc.vector.tensor_copy(
                        pt[:, k * C + j * 128: k * C + (j + 1) * 128], tp[:])

            # ---- main matmul + drain ----
            for i in range(NB):
                orow = outp.tile([128, C], f32, tag="orow")
                for n in range(NN):
                    ps = psum_m.tile([128, 512], f32, tag="mm")
                    for k in range(NK):
                        nc.tensor.matmul(
                            ps[:],
                            xt[:, k * BS + i * 128: k * BS + (i + 1) * 128],
                            pt[:, k * C + n * 512: k * C + (n + 1) * 512],
                            start=(k == 0), stop=(k == NK - 1))
                    dst = orow[:, n * 512:(n + 1) * 512]
                    if n % 2 == 0:
                        nc.scalar.copy(dst, ps[:])
                    else:
                        nc.vector.tensor_copy(dst, ps[:])
                nc.sync.dma_start(
                    out_d[i * 128:(i + 1) * 128, :], orow[:])

    nc.compile()
    return nc


def _get_compiled():
    global _compiled
    if _compiled is None:
        _compiled = _build()
    return _compiled


def kernel(inputs, proto, scalar, _trace=False, **_tr_kw):
    from concourse.bass_utils import run_bass_kernel_spmd

    nc = _get_compiled()
    inputs = np.ascontiguousarray(inputs, dtype=np.float32)
    proto = np.ascontiguousarray(proto, dtype=np.float32)
    sc = np.asarray(scalar, dtype=np.float32).reshape(1, 1)
    ident = np.eye(128, dtype=np.float32)

    in_maps = []
    for c in range(NCORES):
        in_maps.append({
            "x": inputs[c * BS:(c + 1) * BS],
            "proto": proto,
            "scalar": sc,
            "identity": ident,
        })
    res = run_bass_kernel_spmd(nc, in_maps, core_ids=list(range(NCORES)),
                               trace=_trace, **_tr_kw)
    out = np.concatenate([res.results[c]["out"] for c in range(NCORES)], axis=0)
    if _trace:
        kernel.last_results = res
    return out


# revision 6
# speedup vs baseline: 1.0968x; 1.0968x over previous
"""TRN2 Bass kernel for nn_CosClassifier: sim = 10*scalar * cos_sim(inputs, proto).

Data-parallel over 8 NeuronCores: each core computes a (2048, 4096) slab of the
(16384, 4096) similarity matrix. Per core:
  1. DMA in x-slab (2048,256) in 2x1MB groups, proto (4096,256) in 4x1MB
     groups, scalar; groups pipeline so compute starts ~4us in.
  2. Row norms via ACT Square+accum -> Sqrt; DVE reciprocal.
  3. Scale x rows by 10/||x||, proto rows by scalar/||p|| (DVE tensor_scalar).
  4. PE-transpose scaled operands 128x128-blockwise; the PSUM->SBUF copy
     (alternating ACT/DVE) casts to float32r (TF32-like) so the main matmul
     runs at 1 cycle/row.
  5. dots matmul in fp32r (k-alternating lhsT; same-lhsT b2b fp32r is
     pathologically slow), fp32 accumulate in PSUM (one shared 8-bank pool);
     PSUM->SBUF drains alternate ACT/DVE; 1MB contiguous half-row DMAs out.
"""
import sys

sys.path.insert(0, "/opt/trn_rl_repo")

import numpy as np

B, C, D = 16384, 4096, 256
NCORES = 8
BS = B // NCORES          # 2048 rows per core
NB = BS // 128            # 16 b-tiles per core
NCT = C // 128            # 32 c-tiles (proto rows)
NK = D // 128             # 2 k-tiles
NN = C // 512             # 8 n-blocks of 512
XG = 2                    # x groups (1MB each)
PG = 4                    # proto groups (1MB each)
XGT = NB // XG            # 8 b-tiles per x group
PGT = NCT // PG           # 8 c-tiles per p group

_compiled = None


def _build():
    import concourse.bacc as bacc
    import concourse.mybir as mybir
    import concourse.tile as tile

    f32 = mybir.dt.float32
    f32r = mybir.dt.float32r
    Act = mybir.ActivationFunctionType

    nc = bacc.Bacc("TRN2", target_bir_lowering=False, debug=False,
                   num_devices=NCORES)

    x_d = nc.dram_tensor("x", [BS, D], f32, kind="ExternalInput").ap()
    p_d = nc.dram_tensor("proto", [C, D], f32, kind="ExternalInput").ap()
    s_d = nc.dram_tensor("scalar", [1, 1], f32, kind="ExternalInput").ap()
    id_d = nc.dram_tensor("identity", [128, 128], f32, kind="ExternalInput").ap()
    out_d = nc.dram_tensor("out", [BS, C], f32, kind="ExternalOutput").ap()

    with tile.TileContext(nc) as tc:
        with tc.tile_pool(name="sbuf", bufs=1) as pool, \
             tc.tile_pool(name="outp", bufs=4) as outp, \
             tc.tile_pool(name="psum_t", bufs=2, space="PSUM") as psum_t, \
             tc.tile_pool(name="psum_m", bufs=6, space="PSUM") as psum:

            ident = pool.tile([128, 128], f32, tag="ident")
            nc.sync.dma_start(ident[:], id_d[:, :])

            sc = pool.tile([1, 1], f32, tag="sc")
            nc.sync.dma_start(sc[:], s_d[:, :])
            sc_b = pool.tile([128, 1], f32, tag="sc_b")
            nc.gpsimd.partition_broadcast(sc_b[:], sc[:])

            # ---- group loads ----
            x_r = x_d.rearrange("(n p) d -> p n d", p=128)       # [128, NB, 256]
            p_r = p_d.rearrange("(n p) d -> p n d", p=128)       # [128, NCT, 256]
            xg = []
            for g in range(XG):
                t = pool.tile([128, XGT * D], f32, tag=f"xg{g}")
                nc.sync.dma_start(
                    t[:].rearrange("p (n d) -> p n d", d=D),
                    x_r[:, g * XGT:(g + 1) * XGT, :])
                xg.append(t)
            pg = []
            for g in range(PG):
                t = pool.tile([128, PGT * D], f32, tag=f"pg{g}")
                nc.sync.dma_start(
                    t[:].rearrange("p (n d) -> p n d", d=D),
                    p_r[:, g * PGT:(g + 1) * PGT, :])
                pg.append(t)

            # transposed operands (f32r)
            # xt: k-block k at cols k*BS, b-tile i at +i*128
            xt = pool.tile([128, NK * BS], f32r, tag="xt")
            # pt: k-block k at cols k*C, c-tile j at +j*128
            pt = pool.tile([128, NK * C], f32r, tag="pt")

            copy_flip = [0]

            def process_group(grp, gi, n_tiles, with_scalar, dst, dst_stride):
                # norms + scale + transpose for one loaded group tile
                for t in range(n_tiles):
                    src = grp[:, t * D:(t + 1) * D]
                    ssq = pool.tile([128, 1], f32, tag=f"ssq{t % 2}")
                    sq_scr = pool.tile([128, D], f32, tag=f"sqscr{t % 2}")
                    nc.scalar.activation(sq_scr[:], src, Act.Square,
                                         accum_out=ssq[:])
                    nrm = pool.tile([128, 1], f32, tag=f"nrm{t % 2}")
                    # x: sqrt(ssq)/10 (folds *10); proto: plain norm
                    nc.scalar.activation(nrm[:], ssq[:], Act.Sqrt,
                                         scale=1.0 if with_scalar else 0.01)
                    inv = pool.tile([128, 1], f32, tag=f"inv{t % 2}")
                    nc.vector.reciprocal(inv[:], nrm[:])
                    if with_scalar:
                        nc.vector.tensor_mul(inv[:], inv[:], sc_b[:])
                    nc.vector.tensor_scalar_mul(src, src, inv[:])
                for t in range(n_tiles):
                    gt = gi * n_tiles + t   # global tile index
                    for k in range(NK):
                        tp = psum_t.tile([128, 128], f32, tag="tp")
                        nc.tensor.transpose(
                            tp[:],
                            grp[:, t * D + k * 128: t * D + (k + 1) * 128],
                            ident[:])
                        cdst = dst[:, k * dst_stride + gt * 128:
                                   k * dst_stride + (gt + 1) * 128]
                        if copy_flip[0] % 2 == 0:
                            nc.scalar.copy(cdst, tp[:])
                        else:
                            nc.vector.tensor_copy(cdst, tp[:])
                        copy_flip[0] += 1

            # interleave: x group 0, then proto groups, then x group 1
            process_group(xg[0], 0, XGT, False, xt, BS)
            for g in range(PG):
                process_group(pg[g], g, PGT, True, pt, C)
            process_group(xg[1], 1, XGT, False, xt, BS)

            # ---- main matmul + drain ----
            for i in range(NB):
                oh0 = outp.tile([128, C // 2], f32, tag="oh0")
                oh1 = outp.tile([128, C // 2], f32, tag="oh1")
                oh = [oh0, oh1]
                for n in range(NN):
                    ps = psum.tile([128, 512], f32, tag="mm")
                    for k in range(NK):
                        nc.tensor.matmul(
                            ps[:],
                            xt[:, k * BS + i * 128: k * BS + (i + 1) * 128],
                            pt[:, k * C + n * 512: k * C + (n + 1) * 512],
                            start=(k == 0), stop=(k == NK - 1))
                    dst = oh[n // 4][:, (n % 4) * 512:(n % 4 + 1) * 512]
                    if n % 2 == 0:
                        nc.scalar.copy(dst, ps[:])
                    else:
                        nc.vector.tensor_copy(dst, ps[:])
                for h in range(2):
                    nc.sync.dma_start(
                        out_d[i * 128:(i + 1) * 128,
                              h * (C // 2):(h + 1) * (C // 2)], oh[h][:])

    nc.compile()
    return nc


def _get_compiled():
    global _compiled
    if _compiled is None:
        _compiled = _build()
    return _compiled


def kernel(inputs, proto, scalar, _trace=False, **_tr_kw):
    from concourse.bass_utils import run_bass_kernel_spmd

    nc = _get_compiled()
    inputs = np.ascontiguousarray(inputs, dtype=np.float32)
    proto = np.ascontiguousarray(proto, dtype=np.float32)
    sc = np.asarray(scalar, dtype=np.float32).reshape(1, 1)
    ident = np.eye(128, dtype=np.float32)

    in_maps = []
    for c in range(NCORES):
        in_maps.append({
            "x": inputs[c * BS:(c + 1) * BS],
            "proto": proto,
            "scalar": sc,
            "identity": ident,
        })
    res = run_bass_kernel_spmd(nc, in_maps, core_ids=list(range(NCORES)),
                               trace=_trace, **_tr_kw)
    out = np.concatenate([res.results[c]["out"] for c in range(NCORES)], axis=0)
    if _trace:
        kernel.last_results = res
    return out
